# revision 40
# baseline (speedup 1.0000x reference)
"""Trainium2 Bass kernel for nn_CosineSimilarityLayer.

out = l2norm_rows(x) @ l2norm_rows_over_N(W)       x:[4096,512]  W:[512,5994]

Math:  out[b,n] = xscale[b] * sum_d x[b,d] * wscale[d] * W[d,n]
  xscale[b] = rsqrt(max(sum_d x[b,d]^2, eps))   (folded into PSUM eviction)
  wscale[d] = rsqrt(max(sum_n W[d,n]^2, eps))   (folded into transposed x)

Sharding: data-parallel over batch - 8 cores x [512, 512] x-shards, W
replicated.  No collectives: a measured 2KB AllReduce costs ~40us here.

wscale gates every matmul, so the W-norm scan is split three ways (spans
sized from measured engine rates), each consumer fed by its own slice of
a 3MB fp8 shadow in DMA arrival order:
  * ACT fused-Square+accums 1600 columns (its [D,n] fp8 slice lands first),
  * DVE squares+reduces 554 columns (2-pass),
  * the PE covers the remaining 3840 columns as a Gram diagonal:
    DoubleRow fp8 matmuls accumulate diag blocks of W8T^T @ W8T over 15
    row-tile pairs, chasing the shadow DMA at line rate.
fp8 norm error is ~1e-3 relative on wscale -> ~1e-4 on out (gate 2e-2).

x arrives ONLY as host-transposed xT (bf16): the matmul stationary is a
direct DMA, and xscale comes from a second tiny PE gram over xT itself
(bf16 self-products, exact).  PSUM start=True zeroing is bank-granular,
so both gram accumulators use pre-memset banks with start=False.
Diagonals are extracted by identity-mask+accum, split ACT/DVE.

All IO bf16 (host casts, out upcast on host).  Matmul: group-outer /
bt-inner / dt-inner-per-chunk so W chunks are consumed in arrival order
and PSUM stops (and so evictions) spread evenly; leading groups are 1
and 2 chunks so the PE starts on chunk 0; a few warmup matmuls keep the
PE p-state high across the wscale gap.  Eviction (scale by xscale,
round to bf16) alternates ACT/DVE; out DMA on the scalar HWDGE ring.
"""

import os
import sys
import types
from contextlib import ExitStack

import numpy as np


def _ensure_axon_hooks():
    """bass_utils' trace path imports antenv.axon_hooks, which some images
    lack.  Provide it (wired to the ctypes NTFF hook when available) so
    BASS_TRACE=1 profiles instead of crashing.  No-op when already present."""
    try:
        import antenv.axon_hooks  # noqa: F401
        return
    except ImportError:
        pass
    try:
        import antenv
    except ImportError:
        return
    m = types.ModuleType("antenv.axon_hooks")
    holder = {"h": None}
    m.set_axon_ntff_profile_hook = lambda h: holder.__setitem__("h", h)
    m.get_axon_ntff_profile_hook = lambda: holder["h"]
    sys.modules["antenv.axon_hooks"] = m
    antenv.axon_hooks = m
    try:
        from trn_agent_boot.trn_boot import _ntff_profile_via_ctypes
        so = "/opt/axon/libaxon_pjrt.so"
        if os.path.exists(so):
            m.set_axon_ntff_profile_hook(_ntff_profile_via_ctypes(so))
    except Exception:
        pass


_ensure_axon_hooks()

import ml_dtypes
import concourse.bass as bass
import concourse.tile as tile
from concourse import bacc, mybir
from concourse.bass_utils import run_bass_kernel_spmd
from concourse.masks import make_identity

F32 = mybir.dt.float32
BF16 = mybir.dt.bfloat16
FP8 = mybir.dt.float8e4
AF = mybir.ActivationFunctionType
DR = mybir.MatmulPerfMode.DoubleRow

B, D, N = 4096, 512, 5994
NCORES = 8
P = 128
BSH = B // NCORES          # 512 rows of x per core
BT = BSH // P              # 4 b-tiles
DT = D // P                # 4 d-tiles (contraction)
CHUNK = 512                # output n-chunk (one PSUM bank of fp32)
EPS = 1e-12

# ---- W-norm hybrid split (spans from measured engine rates) ----
NPAIR = 15                 # gram row-tile pairs: 15*256 = 3840 rows
NGRAM = NPAIR * 2 * P      # 3840 = W columns covered by the PE gram
NACT = 1600                # ACT's column span (fused Square+accum)
NDVE = N - NGRAM - NACT    # 554, DVE 2-pass
NENG = NACT + NDVE         # engine-shadow columns (2154)

CHUNKS = []
_n0 = 0
while _n0 < N:
    CHUNKS.append((_n0, min(CHUNK, N - _n0)))
    _n0 += CHUNK
NCH = len(CHUNKS)          # 12
# small leading groups (PE starts on chunk 0 alone) and a small final
# group (short tail); 3-chunk groups in between
_GIDX = [[0], [1, 2], [3, 4, 5], [6, 7, 8], [9, 10], [11]]
GROUPS = []                # (start, width, chunk indices)
for _ix in _GIDX:
    _c = [CHUNKS[i] for i in _ix]
    GROUPS.append((_c[0][0], _c[-1][0] + _c[-1][1] - _c[0][0], _ix))


def _build():
    nc = bacc.Bacc("TRN2", target_bir_lowering=False, debug=False,
                   num_devices=NCORES)

    xt_d = nc.dram_tensor("xT", [D, BSH], BF16, kind="ExternalInput").ap()
    w16_d = nc.dram_tensor("W16", [D, N], BF16, kind="ExternalInput").ap()
    w8t_d = nc.dram_tensor("W8T", [P, NPAIR, 2, D], FP8,
                           kind="ExternalInput").ap()
    w8e_d = nc.dram_tensor("W8E", [D, NENG], FP8, kind="ExternalInput").ap()
    o_d = nc.dram_tensor("out", [BSH, N], BF16, kind="ExternalOutput").ap()

    xt_r = xt_d.rearrange("(t p) b -> p t b", p=P)      # [128, 4, 512]
    w16_r = w16_d.rearrange("(t p) n -> p t n", p=P)    # [128, 4, 5994]
    w8e_r = w8e_d.rearrange("(t p) n -> p t n", p=P)    # [128, 4, 2154]
    o_r = o_d.rearrange("(t p) n -> p t n", p=P)        # [128, 4, 5994]

    with tile.TileContext(nc) as tc, ExitStack() as ctx:
        const = ctx.enter_context(tc.tile_pool(name="const", bufs=1))
        sq = ctx.enter_context(tc.tile_pool(name="sq", bufs=2))
        sc = ctx.enter_context(tc.tile_pool(name="sc", bufs=1))
        xt = ctx.enter_context(tc.tile_pool(name="xt", bufs=1))
        wp = ctx.enter_context(tc.tile_pool(name="wp", bufs=1))
        ostp = ctx.enter_context(tc.tile_pool(name="ostp", bufs=4))
        gp = ctx.enter_context(tc.tile_pool(name="gp", bufs=1, space="PSUM"))
        mm = ctx.enter_context(tc.tile_pool(name="mm", bufs=6, space="PSUM"))

        # ---- input DMAs, issued up front in stream order ----
        # each dma_start costs ~400ns + ~1.7ns/descriptor on the issuing
        # engine, so DMAs are consolidated: 12 input issues total.
        w8e = wp.tile([P, DT, NENG], FP8)
        nc.sync.dma_start(w8e[:, :, :NACT], w8e_r[:, :, :NACT])
        nc.sync.dma_start(w8e[:, :, NACT:], w8e_r[:, :, NACT:])
        xtf = xt.tile([P, DT, BSH], BF16, tag="xtf")
        nc.sync.dma_start(xtf, xt_r)
        w8t = wp.tile([P, NPAIR, 2, D], FP8)
        for j0, j1 in ((0, 5), (5, 10), (10, NPAIR)):
            nc.sync.dma_start(w8t[:, j0:j1], w8t_d[:, j0:j1])
        w16 = wp.tile([P, DT, N], BF16)
        for g0, gw, _ in GROUPS:
            nc.sync.dma_start(w16[:, :, g0:g0 + gw],
                              w16_r[:, :, g0:g0 + gw])

        # ---- psum gram banks pre-zeroed (start=True zeroing is
        # bank-granular and would wipe sibling regions) ----
        gps = gp.tile([P, DT, P], F32, name="gps")
        gpx = gp.tile([P, BT, P], F32, name="gpx")
        nc.vector.memset(gps, 0.0)
        nc.vector.memset(gpx, 0.0)

        # ---- preload both ACT tables before any data lands ----
        dum = sc.tile([P, 2], F32)
        dum2 = sc.tile([P, 2], F32)
        nc.scalar.activation(dum[:, 0:1], dum[:, 1:2], AF.Square)
        nc.scalar.activation(dum2[:, 0:1], dum[:, 0:1], AF.Sqrt)
        identity = const.tile([P, P], BF16)
        make_identity(nc, identity)

        # ---- W norm partials ----
        # slot 0: PE gram diag, slot 1: ACT, slot 2: DVE
        wsqp = sc.tile([P, DT, 3], F32)
        for t in range(DT):
            tra = sq.tile([P, NACT], BF16, tag="tra")
            nc.scalar.activation(tra, w8e[:, t, :NACT], AF.Square,
                                 accum_out=wsqp[:, t, 1:2])
        for t in range(DT):
            trd = sq.tile([P, NDVE], BF16, tag="trd")
            nc.vector.tensor_tensor(trd, w8e[:, t, NACT:], w8e[:, t, NACT:],
                                    mybir.AluOpType.mult)
            nc.vector.reduce_sum(wsqp[:, t, 2:3], trd,
                                 axis=mybir.AxisListType.X)

        # PE x-gram first (xT lands before the gram shadow):
        # sum_d x[b,d]^2 as diag blocks of xT^T @ xT (bf16)
        for td in range(DT):
            for bb in range(BT):
                blk = xtf[:, td, bb * P:(bb + 1) * P]
                nc.tensor.matmul(gpx[:, bb, :], blk, blk,
                                 start=False, stop=(td == DT - 1),
                                 skip_group_check=True)

        # PE W-gram: chases the W8T DMA slices
        for j in range(NPAIR):
            for db in range(DT):
                blk = w8t[:, j, :, db * P:(db + 1) * P]
                nc.tensor.matmul(gps[:, db, :], blk, blk, perf_mode=DR,
                                 start=False, stop=(j == NPAIR - 1),
                                 skip_group_check=True)

        # PE warmup: keep the p-state high between the grams and the
        # main burst (results are discarded)
        wu = mm.tile([P, CHUNK], F32, tag="ps", name="wu")
        for k in range(8):
            j = NPAIR - 1 - (k % 2)
            blk = w8t[:, j, :, (k % DT) * P:((k % DT) + 1) * P]
            nc.tensor.matmul(wu[:, :P], blk, blk, perf_mode=DR,
                             start=True, stop=True, skip_group_check=True)

        # ---- diag extracts: DVE masks with identity, ACT row-accums the
        # masked matrix (fused Copy+accum).  x first (its gram finishes
        # first); the eps-max of the reference is skipped: the sums are
        # ~512 and ~5994, never within 9 orders of eps.
        xsq = sc.tile([P, BT], F32)
        for bb in range(BT):
            dx = sq.tile([P, P], F32, tag="diag", name=f"dx{bb}")
            nc.vector.tensor_tensor(dx, gpx[:, bb, :], identity,
                                    mybir.AluOpType.mult)
            tx8 = sq.tile([P, P], BF16, tag="tr8", name=f"tx{bb}")
            nc.scalar.activation(tx8, dx, AF.Copy,
                                 accum_out=xsq[:, bb:bb + 1])
        for db in range(DT):
            dg = sq.tile([P, P], F32, tag="diag", name=f"dg{db}")
            nc.vector.tensor_tensor(dg, gps[:, db, :], identity,
                                    mybir.AluOpType.mult)
            tr8 = sq.tile([P, P], BF16, tag="tr8", name=f"tr{db}")
            nc.scalar.activation(tr8, dg, AF.Copy,
                                 accum_out=wsqp[:, db, 0:1])

        wsq = sc.tile([P, DT, 1], F32)
        nc.vector.reduce_sum(wsq, wsqp, axis=mybir.AxisListType.X)
        wsr = sc.tile([P, DT, 1], F32)
        nc.scalar.sqrt(wsr, wsq)
        wsc = sc.tile([P, DT, 1], F32)
        nc.vector.reciprocal(wsc, wsr)

        # ---- fold wscale into the host-transposed x ----
        xtr = xt.tile([P, DT, BSH], BF16, tag="xtr")
        for dt in range(DT):
            nc.vector.tensor_scalar_mul(xtr[:, dt, :], xtf[:, dt, :],
                                        wsc[:, dt, :])

        xsr = sc.tile([P, BT], F32)
        nc.scalar.sqrt(xsr, xsq)
        xsc = sc.tile([P, BT], F32)
        nc.vector.reciprocal(xsc, xsr)

        # ---- matmul: group outer (W arrival order), bt inner; dt outer
        # within a group so the stationary is reused across banks.
        # Output staged per group across all bt so there is ONE out DMA
        # per group (issue cost), on the scalar HWDGE ring. ----
        evict = [0]
        for g, (g0, gw, gix) in enumerate(GROUPS):
            grp = [CHUNKS[i] for i in gix]
            ost = ostp.tile([P, BT, 3 * CHUNK], BF16, tag="ost")
            for bt in range(BT):
                pss = [mm.tile([P, CHUNK], F32, tag="ps", name=f"ps{c}")
                       for c in range(len(grp))]
                for dt in range(DT):
                    for c, (n0, nw) in enumerate(grp):
                        nc.tensor.matmul(
                            pss[c][:, :nw],
                            xtr[:, dt, bt * P:(bt + 1) * P],
                            w16[:, dt, n0:n0 + nw],
                            start=(dt == 0), stop=(dt == DT - 1))
                for c, (n0, nw) in enumerate(grp):
                    # GPSIMD cannot read PSUM: alternate ACT/DVE.
                    dst = ost[:, bt, n0 - g0:n0 - g0 + nw]
                    if evict[0] % 2 == 0:
                        nc.scalar.activation(dst, pss[c][:, :nw], AF.Copy,
                                             scale=xsc[:, bt:bt + 1])
                    else:
                        nc.vector.tensor_scalar_mul(dst, pss[c][:, :nw],
                                                    xsc[:, bt:bt + 1])
                    evict[0] += 1
            nc.scalar.dma_start(o_r[:, :, g0:g0 + gw], ost[:, :, :gw])

    nc.compile()
    return nc


LAST_RESULT = None


def kernel(x: np.ndarray, W: np.ndarray) -> np.ndarray:
    global LAST_RESULT
    x = np.ascontiguousarray(x, dtype=np.float32)
    W = np.ascontiguousarray(W, dtype=np.float32)
    assert x.shape == (B, D) and W.shape == (D, N)

    x16 = x.astype(ml_dtypes.bfloat16)
    W16 = np.ascontiguousarray(W.astype(ml_dtypes.bfloat16))
    W8E = np.ascontiguousarray(W[:, NGRAM:].astype(ml_dtypes.float8_e4m3))

    # gram shadow: W^T rows [0:NGRAM] interleaved to [128, pair, 2, D]:
    # partition p of pair j holds rows 256j+p and 256j+128+p.
    w8t = W.T[:NGRAM].astype(ml_dtypes.float8_e4m3)
    w8t = np.ascontiguousarray(
        w8t.reshape(NPAIR, 2, P, D).transpose(2, 0, 1, 3))

    nc = _build()

    in_maps = []
    for c in range(NCORES):
        xs = x16[c * BSH:(c + 1) * BSH]
        in_maps.append({"xT": np.ascontiguousarray(xs.T),
                        "W16": W16, "W8T": w8t, "W8E": W8E})

    res = run_bass_kernel_spmd(nc, in_maps, core_ids=list(range(NCORES)))
    LAST_RESULT = res
    out = np.concatenate([res.results[c]["out"] for c in range(NCORES)],
                         axis=0)
    return out.astype(np.float32)
